# revision 2
# baseline (speedup 1.0000x reference)
"""Trainium2 Bass kernel for nn_BuildCost (light-field cost volume), v3.

out[b, co, d, i, j] = (1/mask_avg[i,j]) * sum_{p,q} W[co, p*9+q]
                       * mask[p*9+q, i, j] * x[b, co//4, p*9+q, i+dd*(4-p), j+dd*(4-q)]

Design: pure streaming grouped-GEMM at the modeled HBM roofline.  Host
prep performs the im2col-style unfold (the 9-disparity shear), folds the
mask modulation and 1/mask_avg normalization into the unfolded operand
(cf. x_pad prep in the sharding hint), and quantizes it to fp8-e3m4
(4 mantissa bits; |xm| <= ~12 < 15.5 so no clipping in practice),
halving wire bytes vs bf16 at measured 1.75e-2 max rel err (gate 2e-2).
Weights stay bf16 (mixed-dtype matmul, validated on hw).

Per (half-band, disparity) the device streams 21 contraction chunks
([<=128 k-rows x 2304 px]) of the block-diagonal grouped-conv matmul,
accumulated in PSUM across chunks in five bank-aligned 512-col regions
(start= only on each bank's first matmul: start pend-zeroes the whole
2KB zero-region).

Sharding: 8 cores x 24-row output bands.  Per core: ~107 MB fp8 in +
~10 MB bf16 out = ~330 us DMA at the modeled 360 GB/s; PE ~363 us at
full clock.  PE/DMA co-bound by design (ridge regime).
"""

import sys

sys.path.insert(0, "/opt/trn_rl_repo")

import numpy as np
import ml_dtypes

A = 9
C0 = 4
BDR = 16
H = W_IMG = 192
CIN = 32
COUT = 128
ND = 9
N_CORES = 8
BAND = H // N_CORES          # 24 output rows per core
HALF = BAND // 2             # 12 rows per half-band
NPIX = HALF * W_IMG          # 2304 pixels per half-band
NHD = 2 * ND                 # (half, d) instances, hd = half*9 + d
ROWS = A * A * CIN           # 2592 contraction rows per (half, d)
NPAIR = 10                   # 10 pairs of full 128-row chunks
NCHUNK = 21                  # 20 full chunks + 32-row tail
TAILK = 32                   # leftover chunk (view 80, 32 channels)
REGW = [512, 512, 512, 512, 256]          # PSUM accumulation regions
REG0 = [0, 512, 1024, 1536, 2048]

_E3 = ml_dtypes.float8_e3m4
_BF = ml_dtypes.bfloat16
_PROGRAM = None


def _build_program():
    import concourse.bacc as bacc
    import concourse.tile as tile
    import concourse.bass as bass
    from concourse import mybir

    nc = bacc.Bacc("TRN2", target_bir_lowering=False, debug=False,
                   num_devices=N_CORES)

    xmd = nc.dram_tensor("xm8", [NHD, ROWS, NPIX], mybir.dt.float8e3,
                         kind="ExternalInput").ap()
    wtd = nc.dram_tensor("wt16", [128, NCHUNK * 128], mybir.dt.bfloat16,
                         kind="ExternalInput").ap()
    od = nc.dram_tensor("out", [COUT, ND, 2, NPIX], mybir.dt.bfloat16,
                        kind="ExternalOutput").ap()

    with tile.TileContext(nc) as tc:
        with (
            tc.tile_pool(name="wpool", bufs=1) as wpool,
            tc.tile_pool(name="xpool", bufs=6) as xpool,
            tc.tile_pool(name="xtpool", bufs=2) as xtpool,
            tc.tile_pool(name="opool", bufs=4) as opool,
            tc.tile_pool(name="psum", bufs=7, space="PSUM") as pspool,
        ):
            w_all = wpool.tile([128, NCHUNK * 128], mybir.dt.bfloat16,
                               name="w_all")
            nc.sync.dma_start(out=w_all[:], in_=wtd[:])

            for half in range(2):
                for d in range(ND):
                    hd = half * ND + d
                    xts = []
                    for j in range(NPAIR):
                        xt = xpool.tile([128, 2, NPIX], mybir.dt.float8e3,
                                        tag="xt")
                        eng = nc.sync if (hd * NPAIR + j) % 2 == 0 else nc.scalar
                        eng.dma_start(
                            out=xt[:],
                            in_=bass.AP(
                                tensor=xmd.tensor,
                                offset=(hd * ROWS + 256 * j) * NPIX,
                                ap=[[NPIX, 128], [128 * NPIX, 2], [1, NPIX]]))
                        xts.append(xt)
                    xtl = xtpool.tile([TAILK, NPIX], mybir.dt.float8e3,
                                      tag="xtl")
                    nc.sync.dma_start(
                        out=xtl[:],
                        in_=bass.AP(tensor=xmd.tensor,
                                    offset=(hd * ROWS + 2560) * NPIX,
                                    ap=[[NPIX, TAILK], [1, NPIX]]))

                    pss = []
                    for r in range(5):
                        ps = pspool.tile([128, 512], mybir.dt.float32,
                                         tag="ps")
                        pss.append(ps)

                    for c in range(NCHUNK):
                        if c < 2 * NPAIR:
                            wv = w_all[:, c * 128:(c + 1) * 128]
                            xv = xts[c // 2][:, c % 2, :]
                        else:
                            wv = w_all[:TAILK, c * 128:(c + 1) * 128]
                            xv = xtl[:]
                        for r in range(5):
                            n0, nw = REG0[r], REGW[r]
                            nc.tensor.matmul(
                                pss[r][:, :nw],
                                wv,
                                xv[:, n0:n0 + nw],
                                start=(c == 0),
                                stop=(c == NCHUNK - 1))
                    for r in range(5):
                        n0, nw = REG0[r], REGW[r]
                        osb = opool.tile([128, 512], mybir.dt.bfloat16,
                                         tag="osb")
                        nc.scalar.copy(osb[:, :nw], pss[r][:, :nw])
                        nc.gpsimd.dma_start(out=od[:, d, half, n0:n0 + nw],
                                            in_=osb[:, :nw])

    nc.compile()
    return nc


def _get_program():
    global _PROGRAM
    if _PROGRAM is None:
        _PROGRAM = _build_program()
    return _PROGRAM


def _host_prep(x, mask, W):
    x = np.asarray(x, dtype=np.float32)
    mask = np.asarray(mask, dtype=np.float32)
    W = np.asarray(W, dtype=np.float32)

    mask_n = mask[0] / mask[0].mean(axis=0, keepdims=True)   # [81,192,192]
    xv = np.ascontiguousarray(x[0].transpose(1, 0, 2, 3))    # [81,32,192,192]
    xp = np.zeros((A * A, CIN, H + 2 * BDR, W_IMG + 2 * BDR),
                  dtype=np.float32)
    xp[:, :, BDR:BDR + H, BDR:BDR + W_IMG] = xv

    # block-diagonal grouped-conv weights, packed [128 k, NCHUNK*128]
    co = np.arange(COUT)
    g = co // (COUT // CIN)
    wt = np.zeros((128, NCHUNK * 128), dtype=np.float32)
    for c in range(NCHUNK):
        for l in range(4 if c < 2 * NPAIR else 1):
            v = 4 * c + l
            wt[l * 32 + g, c * 128 + co] = W[co, v]
    wt16 = wt.astype(_BF)

    # unfolded, mask-modulated, fp8-e3m4 operand: [core, hd, ROWS, NPIX]
    xm8 = np.zeros((N_CORES, NHD, ROWS, NPIX), dtype=_E3)
    for d in range(ND):
        dd = d - 4
        for p in range(A):
            rs = BDR + dd * (C0 - p)
            for q in range(A):
                v = p * A + q
                cs = BDR + dd * (C0 - q)
                xs = xp[v, :, rs:rs + H, cs:cs + W_IMG]       # [32,192,192]
                xmq = np.clip(mask_n[v][None] * xs,
                              -15.5, 15.5).astype(_E3)        # [32,192,192]
                for k in range(N_CORES):
                    for half in range(2):
                        r0 = BAND * k + HALF * half
                        xm8[k, half * ND + d, v * CIN:(v + 1) * CIN, :] = \
                            xmq[:, r0:r0 + HALF, :].reshape(CIN, NPIX)
    in_maps = [{"xm8": xm8[k], "wt16": wt16} for k in range(N_CORES)]
    return in_maps


PROFILE = False
LAST_RESULTS = None


def kernel(x, mask, W):
    global LAST_RESULTS
    from concourse.bass_utils import run_bass_kernel_spmd

    nc = _get_program()
    in_maps = _host_prep(x, mask, W)
    res = run_bass_kernel_spmd(nc, in_maps, list(range(N_CORES)),
                               trace=PROFILE)
    LAST_RESULTS = res

    out = np.empty((1, COUT, ND, H, W_IMG), dtype=np.float32)
    for k in range(N_CORES):
        ob = res.results[k]["out"].astype(np.float32)   # [128, 9, 2, 2304]
        out[0, :, :, BAND * k:BAND * k + BAND, :] = \
            ob.reshape(COUT, ND, BAND, W_IMG)
    return out


# revision 3
# speedup vs baseline: 1.0572x; 1.0572x over previous
"""Trainium2 Bass kernel for nn_BuildCost (light-field cost volume), v4.

out[b, co, d, i, j] = (1/mask_avg[i,j]) * sum_{p,q} W[co, p*9+q]
                       * mask[p*9+q, i, j] * x[b, co//4, p*9+q, i+dd*(4-p), j+dd*(4-q)]

Pure streaming grouped-GEMM at the modeled HBM/PE ridge.  Host prep
performs the im2col-style unfold (9-disparity shear), folds the mask
modulation and 1/mask_avg normalization into the unfolded operand
(cf. x_pad prep in the sharding hint), and quantizes to fp8-e3m4
(4 mantissa bits; measured 1.75e-2 max rel err vs the 2e-2 gate).
Weights stay bf16 (mixed-dtype matmul, hw-validated).

Per (half-band, disparity): 80 views stream as 20 x 128-row chunks of
the block-diagonal grouped-conv matmul, PSUM-accumulated in five
bank-aligned 512-col regions (start= pend-zeroes the whole 2KB region,
so regions never share banks).  The 81st view's contribution is rank-1
per output row (out[co] += W[co,80] * xm[co//4]), so it skips the PE:
the host pre-folds W into its fp8 operand, replicated to the 128 cout
rows, and the otherwise-idle DVE applies it during the PSUM drain
(osb = psum + xtr).  This removes a full 128-row stream from the PE
(-17 us) for +5 MB of DMA on the slack side of the roofline.

Sharding: 8 cores x 24-row output bands.  Per core ~111 MB fp8 in,
~10.6 MB bf16 out: DMA ~344 us modeled, PE ~346 us: ridge-balanced.
"""

import sys

sys.path.insert(0, "/opt/trn_rl_repo")

import numpy as np
import ml_dtypes

A = 9
C0 = 4
BDR = 16
H = W_IMG = 192
CIN = 32
COUT = 128
ND = 9
N_CORES = 8
BAND = H // N_CORES          # 24 output rows per core
HALF = BAND // 2             # 12 rows per half-band
NPIX = HALF * W_IMG          # 2304 pixels per half-band
NHD = 2 * ND                 # (half, d) instances, hd = half*9 + d
NPAIR = 10                   # 10 pairs of 128-row chunks (views 0..79)
NCHUNK = 20                  # PE chunks (views 0..79)
ROWS = NCHUNK * 128          # 2560 PE contraction rows per (half, d)
REGW = [512, 512, 512, 512, 256]          # PSUM accumulation regions
REG0 = [0, 512, 1024, 1536, 2048]

_E3 = ml_dtypes.float8_e3m4
_BF = ml_dtypes.bfloat16
_PROGRAM = None


def _build_program():
    import concourse.bacc as bacc
    import concourse.tile as tile
    import concourse.bass as bass
    from concourse import mybir

    nc = bacc.Bacc("TRN2", target_bir_lowering=False, debug=False,
                   num_devices=N_CORES)

    xmd = nc.dram_tensor("xm8", [NHD, ROWS, NPIX], mybir.dt.float8e3,
                         kind="ExternalInput").ap()
    xtd = nc.dram_tensor("xtr8", [NHD, 128, NPIX], mybir.dt.float8e3,
                         kind="ExternalInput").ap()
    wtd = nc.dram_tensor("wt16", [128, NCHUNK * 128], mybir.dt.bfloat16,
                         kind="ExternalInput").ap()
    od = nc.dram_tensor("out", [COUT, ND, 2, NPIX], mybir.dt.bfloat16,
                        kind="ExternalOutput").ap()

    with tile.TileContext(nc) as tc:
        with (
            tc.tile_pool(name="wpool", bufs=1) as wpool,
            tc.tile_pool(name="xpool", bufs=6) as xpool,
            tc.tile_pool(name="xtpool", bufs=3) as xtpool,
            tc.tile_pool(name="opool", bufs=4) as opool,
            tc.tile_pool(name="psum", bufs=7, space="PSUM") as pspool,
            tc.tile_pool(name="wupsum", bufs=1, space="PSUM") as wupool,
        ):
            w_all = wpool.tile([128, NCHUNK * 128], mybir.dt.bfloat16,
                               name="w_all")
            nc.gpsimd.dma_start(out=w_all[:], in_=wtd[:])

            # PE p-state warmup during the initial DMA fill
            wu = wpool.tile([128, 512], mybir.dt.float8e3, name="wu")
            nc.vector.memset(wu[:], 0.0)
            wups = wupool.tile([128, 512], mybir.dt.float32, name="wups")
            for _wi in range(10):
                nc.tensor.matmul(wups[:], wu[:, :128], wu[:],
                                 start=True, stop=True)

            for half in range(2):
                for d in range(ND):
                    hd = half * ND + d
                    xts = []
                    for j in range(NPAIR):
                        xt = xpool.tile([128, 2, NPIX], mybir.dt.float8e3,
                                        tag="xt")
                        eng = nc.sync if (hd * NPAIR + j) % 2 == 0 else nc.scalar
                        eng.dma_start(
                            out=xt[:],
                            in_=bass.AP(
                                tensor=xmd.tensor,
                                offset=(hd * ROWS + 256 * j) * NPIX,
                                ap=[[NPIX, 128], [128 * NPIX, 2], [1, NPIX]]))
                        xts.append(xt)
                    xtr = xtpool.tile([128, NPIX], mybir.dt.float8e3,
                                      tag="xtr")
                    nc.gpsimd.dma_start(out=xtr[:], in_=xtd[hd])

                    pss = []
                    for r in range(5):
                        ps = pspool.tile([128, 512], mybir.dt.float32,
                                         tag="ps")
                        pss.append(ps)

                    for c in range(NCHUNK):
                        wv = w_all[:, c * 128:(c + 1) * 128]
                        xv = xts[c // 2][:, c % 2, :]
                        for r in range(5):
                            n0, nw = REG0[r], REGW[r]
                            nc.tensor.matmul(
                                pss[r][:, :nw],
                                wv,
                                xv[:, n0:n0 + nw],
                                start=(c == 0),
                                stop=(c == NCHUNK - 1))
                    for r in range(5):
                        n0, nw = REG0[r], REGW[r]
                        osb = opool.tile([128, 512], mybir.dt.bfloat16,
                                         tag="osb")
                        nc.vector.tensor_add(osb[:, :nw], pss[r][:, :nw],
                                             xtr[:, n0:n0 + nw])
                        nc.scalar.dma_start(out=od[:, d, half, n0:n0 + nw],
                                            in_=osb[:, :nw])

    nc.compile()
    return nc


def _get_program():
    global _PROGRAM
    if _PROGRAM is None:
        _PROGRAM = _build_program()
    return _PROGRAM


def _host_prep(x, mask, W):
    x = np.asarray(x, dtype=np.float32)
    mask = np.asarray(mask, dtype=np.float32)
    W = np.asarray(W, dtype=np.float32)

    mask_n = mask[0] / mask[0].mean(axis=0, keepdims=True)   # [81,192,192]
    xv = np.ascontiguousarray(x[0].transpose(1, 0, 2, 3))    # [81,32,192,192]
    xp = np.zeros((A * A, CIN, H + 2 * BDR, W_IMG + 2 * BDR),
                  dtype=np.float32)
    xp[:, :, BDR:BDR + H, BDR:BDR + W_IMG] = xv

    # block-diagonal grouped-conv weights for views 0..79, [128 k, 20*128]
    co = np.arange(COUT)
    g = co // (COUT // CIN)
    wt = np.zeros((128, NCHUNK * 128), dtype=np.float32)
    for c in range(NCHUNK):
        for l in range(4):
            v = 4 * c + l
            wt[l * 32 + g, c * 128 + co] = W[co, v]
    wt16 = wt.astype(_BF)

    # unfolded, mask-modulated, fp8-e3m4 operand: [core, hd, ROWS, NPIX]
    xm8 = np.zeros((N_CORES, NHD, ROWS, NPIX), dtype=_E3)
    # view 80 (p=q=8): W pre-folded, replicated over cout rows
    xtr8 = np.zeros((N_CORES, NHD, 128, NPIX), dtype=_E3)
    for d in range(ND):
        dd = d - 4
        for p in range(A):
            rs = BDR + dd * (C0 - p)
            for q in range(A):
                v = p * A + q
                cs = BDR + dd * (C0 - q)
                xs = xp[v, :, rs:rs + H, cs:cs + W_IMG]       # [32,192,192]
                if v < 80:
                    xmq = np.clip(mask_n[v][None] * xs,
                                  -15.5, 15.5).astype(_E3)
                    for k in range(N_CORES):
                        for half in range(2):
                            r0 = BAND * k + HALF * half
                            xm8[k, half * ND + d,
                                v * CIN:(v + 1) * CIN, :] = \
                                xmq[:, r0:r0 + HALF, :].reshape(CIN, NPIX)
                else:
                    xw = (mask_n[v][None] * xs)[g] * \
                        W[co, v][:, None, None]               # [128,192,192]
                    xwq = np.clip(xw, -15.5, 15.5).astype(_E3)
                    for k in range(N_CORES):
                        for half in range(2):
                            r0 = BAND * k + HALF * half
                            xtr8[k, half * ND + d] = \
                                xwq[:, r0:r0 + HALF, :].reshape(128, NPIX)
    in_maps = [{"xm8": xm8[k], "xtr8": xtr8[k], "wt16": wt16}
               for k in range(N_CORES)]
    return in_maps


PROFILE = False
LAST_RESULTS = None


def kernel(x, mask, W):
    global LAST_RESULTS
    from concourse.bass_utils import run_bass_kernel_spmd

    nc = _get_program()
    in_maps = _host_prep(x, mask, W)
    res = run_bass_kernel_spmd(nc, in_maps, list(range(N_CORES)),
                               trace=PROFILE)
    LAST_RESULTS = res

    out = np.empty((1, COUT, ND, H, W_IMG), dtype=np.float32)
    for k in range(N_CORES):
        ob = res.results[k]["out"].astype(np.float32)   # [128, 9, 2, 2304]
        out[0, :, :, BAND * k:BAND * k + BAND, :] = \
            ob.reshape(COUT, ND, BAND, W_IMG)
    return out


# revision 4
# speedup vs baseline: 1.0669x; 1.0091x over previous
"""Trainium2 Bass kernel for nn_BuildCost, v7: coordinated-rounding e4m3 + DoubleRow.

Same streaming grouped-GEMM dataflow as v4 (host im2col unfold + mask fold,
fp8 operand, 8 cores x 24-row bands, five bank-aligned PSUM regions), with
two changes that halve the PE time:

1. The unfolded operand is quantized to fp8-e4m3 with WEIGHTED COORDINATED
   ROUNDING: per (channel-group, d, pixel), the 81 views' round-up/down
   choices are picked greedily (then 2 flip-refinement sweeps) to minimize
   the max over the group's 4 couts of the running weighted error
   sum_v W[co,v]*eps_v.  Measured worst rel err 1.14e-2 vs 3.2e-2 for
   independent rounding (gate 2e-2) - e4m3 becomes usable.

2. With both operands e4m3, chunk pairs run as DoubleRow matmuls
   (0.5 cyc/out-col) twice: once with e4m3 weights W8, once with the e4m3
   residual (W - W8), cancelling weight-quantization error.  The 32-row
   tail (view 80) stays a plain mixed-dtype matmul with exact bf16 weights.

PE ~190 us, DMA ~330 us: DMA-bound.  start= flags: only the first matmul
touching each PSUM bank (start pend-zeroes the whole 2KB zero-region).
"""

import sys

sys.path.insert(0, "/opt/trn_rl_repo")

import numpy as np
import ml_dtypes

A = 9
C0 = 4
BDR = 16
H = W_IMG = 192
CIN = 32
COUT = 128
ND = 9
N_CORES = 8
BAND = H // N_CORES
HALF = BAND // 2
NPIX = HALF * W_IMG          # 2304
NHD = 2 * ND
NPAIR = 10                   # views 0..79 as 10 DR pairs
ROWS = 81 * CIN              # 2592 rows: views 0..80
TAILK = 32
W8COLS = NPAIR * 2 * 2 * 128   # W8 pairs | Wr8 pairs
REGW = [512, 512, 512, 512, 256]
REG0 = [0, 512, 1024, 1536, 2048]

_E4 = ml_dtypes.float8_e4m3
_BF = ml_dtypes.bfloat16
_PROGRAM = None


def _build_program():
    import concourse.bacc as bacc
    import concourse.tile as tile
    import concourse.bass as bass
    from concourse import mybir

    nc = bacc.Bacc("TRN2", target_bir_lowering=False, debug=False,
                   num_devices=N_CORES)

    xmd = nc.dram_tensor("xm8", [NHD, ROWS, NPIX], mybir.dt.float8e4,
                         kind="ExternalInput").ap()
    w8d = nc.dram_tensor("w8", [128, W8COLS], mybir.dt.float8e4,
                         kind="ExternalInput").ap()
    wtd = nc.dram_tensor("wtail", [TAILK, 128], mybir.dt.bfloat16,
                         kind="ExternalInput").ap()
    od = nc.dram_tensor("out", [COUT, ND, 2, NPIX], mybir.dt.bfloat16,
                        kind="ExternalOutput").ap()

    DR = mybir.MatmulPerfMode.DoubleRow

    with tile.TileContext(nc) as tc:
        with (
            tc.tile_pool(name="wpool", bufs=1) as wpool,
            tc.tile_pool(name="xpool", bufs=6) as xpool,
            tc.tile_pool(name="xtpool", bufs=2) as xtpool,
            tc.tile_pool(name="opool", bufs=4) as opool,
            tc.tile_pool(name="psum", bufs=7, space="PSUM") as pspool,
            tc.tile_pool(name="wupsum", bufs=1, space="PSUM") as wupool,
        ):
            w8 = wpool.tile([128, W8COLS], mybir.dt.float8e4, name="w8")
            nc.gpsimd.dma_start(out=w8[:], in_=w8d[:])
            wt = wpool.tile([TAILK, 128], mybir.dt.bfloat16, name="wt")
            nc.gpsimd.dma_start(out=wt[:], in_=wtd[:])

            def wpair(i2, j):           # [128, 2, 128] lhsT
                base = (i2 * NPAIR + j) * 256
                return w8[:, base:base + 256].rearrange(
                    "p (two m) -> p two m", two=2)

            # PE p-state warmup during the DMA fill
            wu = wpool.tile([128, 512], mybir.dt.float8e4, name="wu")
            nc.vector.memset(wu[:], 0.0)
            wups = wupool.tile([128, 512], mybir.dt.float32, name="wups")
            for _wi in range(10):
                nc.tensor.matmul(wups[:], wu[:, :128], wu[:],
                                 start=True, stop=True)

            for half in range(2):
                for d in range(ND):
                    hd = half * ND + d
                    xts = []
                    for j in range(NPAIR):
                        xt = xpool.tile([128, 2, NPIX], mybir.dt.float8e4,
                                        tag="xt")
                        eng = nc.sync if (hd * NPAIR + j) % 2 == 0 else nc.scalar
                        eng.dma_start(
                            out=xt[:],
                            in_=bass.AP(
                                tensor=xmd.tensor,
                                offset=(hd * ROWS + 256 * j) * NPIX,
                                ap=[[NPIX, 128], [128 * NPIX, 2], [1, NPIX]]))
                        xts.append(xt)
                    xtl = xtpool.tile([TAILK, NPIX], mybir.dt.float8e4,
                                      tag="xtl")
                    nc.sync.dma_start(
                        out=xtl[:],
                        in_=bass.AP(tensor=xmd.tensor,
                                    offset=(hd * ROWS + 2560) * NPIX,
                                    ap=[[NPIX, TAILK], [1, NPIX]]))

                    pss = []
                    for r in range(5):
                        ps = pspool.tile([128, 512], mybir.dt.float32,
                                         tag="ps")
                        pss.append(ps)

                    for j in range(NPAIR):
                        for i2 in range(2):     # W8 pass, then residual
                            wv = wpair(i2, j)
                            for r in range(5):
                                n0, nw = REG0[r], REGW[r]
                                for s0 in range(0, nw, 256):
                                    nc.tensor.matmul(
                                        pss[r][:, s0:s0 + 256],
                                        wv,
                                        xts[j][:, :, n0 + s0:n0 + s0 + 256],
                                        start=(j == 0 and i2 == 0 and s0 == 0),
                                        stop=False,
                                        perf_mode=DR)
                    for r in range(5):
                        n0, nw = REG0[r], REGW[r]
                        nc.tensor.matmul(
                            pss[r][:, :nw], wt[:], xtl[:, n0:n0 + nw],
                            start=False, stop=True)
                        osb = opool.tile([128, 512], mybir.dt.bfloat16,
                                         tag="osb")
                        nc.vector.tensor_copy(osb[:, :nw], pss[r][:, :nw])
                        nc.scalar.dma_start(out=od[:, d, half, n0:n0 + nw],
                                            in_=osb[:, :nw])

    nc.compile()
    return nc


def _get_program():
    global _PROGRAM
    if _PROGRAM is None:
        _PROGRAM = _build_program()
    return _PROGRAM


def _neighbors(v):
    """(rtn, other) e4m3 candidates bracketing v, as f32."""
    q = v.astype(_E4)
    qf = q.astype(np.float32)
    bits = q.view(np.uint8).astype(np.int16)
    sign = bits & 0x80
    mag = bits & 0x7F
    up_mag = np.clip(mag + 1, 0, 126)
    dn_mag = np.clip(mag - 1, 0, 126)
    bigger = np.where(sign == 0, sign | up_mag, sign | dn_mag).astype(np.uint8)
    smaller = np.where(sign == 0, sign | dn_mag, sign | up_mag).astype(np.uint8)
    other = np.where(qf < v, bigger.view(_E4).astype(np.float32),
                     np.where(qf > v, smaller.view(_E4).astype(np.float32),
                              qf))
    return qf, other


def _host_prep(x, mask, W):
    x = np.asarray(x, dtype=np.float32)
    mask = np.asarray(mask, dtype=np.float32)
    W = np.asarray(W, dtype=np.float32)

    mask_n = mask[0] / mask[0].mean(axis=0, keepdims=True)
    xv = np.ascontiguousarray(x[0].transpose(1, 0, 2, 3))
    xp = np.zeros((81, CIN, H + 2 * BDR, W_IMG + 2 * BDR), dtype=np.float32)
    xp[:, :, BDR:BDR + H, BDR:BDR + W_IMG] = xv

    co = np.arange(COUT)
    g = co // (COUT // CIN)

    # DR weights: W8 pair passes + Wr8 pair passes
    W8 = W.astype(_E4).astype(np.float32)
    Wr8 = (W - W8).astype(_E4).astype(np.float32)
    w8 = np.zeros((128, W8COLS), dtype=np.float32)
    for i2, Wq in ((0, W8), (1, Wr8)):
        for j in range(NPAIR):
            base = (i2 * NPAIR + j) * 256
            for i in range(2):
                for l in range(4):
                    v = 4 * (2 * j + i) + l
                    w8[l * 32 + g, base + i * 128 + co] = Wq[co, v]
    w8 = w8.astype(_E4)
    wtail = np.zeros((TAILK, 128), dtype=np.float32)
    wtail[g, co] = W[co, 80]
    wtail = wtail.astype(_BF)

    # coordinated-rounding e4m3 quantization, per disparity
    W4 = W.reshape(CIN, 4, 81)
    xm8 = np.zeros((N_CORES, NHD, ROWS, NPIX), dtype=_E4)
    for d in range(ND):
        dd = d - 4
        E = np.zeros((CIN, 4, H, W_IMG), dtype=np.float32)
        cands = {}
        qvals = np.zeros((81, CIN, H, W_IMG), dtype=np.float32)
        errs = np.zeros((81, CIN, H, W_IMG), dtype=np.float32)
        for v in range(81):
            p, q_ = v // A, v % A
            rs, cs = BDR + dd * (C0 - p), BDR + dd * (C0 - q_)
            prod = mask_n[v][None] * xp[v, :, rs:rs + H, cs:cs + W_IMG]
            qa, qb = _neighbors(prod)
            ea, eb = qa - prod, qb - prod
            Wv = W4[:, :, v]
            ca = E + Wv[:, :, None, None] * ea[:, None]
            cb = E + Wv[:, :, None, None] * eb[:, None]
            pick_b = np.abs(cb).max(axis=1) < np.abs(ca).max(axis=1)
            E = np.where(pick_b[:, None], cb, ca)
            cands[v] = (qa, qb, ea, eb)
            qvals[v] = np.where(pick_b, qb, qa)
            errs[v] = np.where(pick_b, eb, ea)
        for _sweep in range(2):
            for v in range(81):
                qa, qb, ea, eb = cands[v]
                Wv = W4[:, :, v]
                Ew = E - Wv[:, :, None, None] * errs[v][:, None]
                ca = Ew + Wv[:, :, None, None] * ea[:, None]
                cb = Ew + Wv[:, :, None, None] * eb[:, None]
                pick_b = np.abs(cb).max(axis=1) < np.abs(ca).max(axis=1)
                E = np.where(pick_b[:, None], cb, ca)
                qvals[v] = np.where(pick_b, qb, qa)
                errs[v] = np.where(pick_b, eb, ea)
        q8 = qvals.astype(_E4)                       # [81, 32, H, W]
        for k in range(N_CORES):
            for half in range(2):
                r0 = BAND * k + HALF * half
                xm8[k, half * ND + d] = \
                    q8[:, :, r0:r0 + HALF, :].reshape(ROWS, NPIX)
    in_maps = [{"xm8": xm8[k], "w8": w8, "wtail": wtail}
               for k in range(N_CORES)]
    return in_maps


PROFILE = False
LAST_RESULTS = None


def kernel(x, mask, W):
    global LAST_RESULTS
    from concourse.bass_utils import run_bass_kernel_spmd

    nc = _get_program()
    in_maps = _host_prep(x, mask, W)
    res = run_bass_kernel_spmd(nc, in_maps, list(range(N_CORES)),
                               trace=PROFILE)
    LAST_RESULTS = res

    out = np.empty((1, COUT, ND, H, W_IMG), dtype=np.float32)
    for k in range(N_CORES):
        ob = res.results[k]["out"].astype(np.float32)
        out[0, :, :, BAND * k:BAND * k + BAND, :] = \
            ob.reshape(COUT, ND, BAND, W_IMG)
    return out
